# revision 1
# baseline (speedup 1.0000x reference)
"""LocallyConnected2d Trainium2 kernel.

out[b,o,p,q] = sum_{i,kh,kw} x[b, i, 2p+kh, 2q+kw] * weight[0, o, i, p, q, kh*3+kw]

Strategy:
- Shard the H' (=31) output-row dimension across 8 cores (4 rows/core,
  core 7 gets a duplicated padding row so the SPMD program is uniform).
  This splits the dominant traffic — the 35.4MB per-location weight —
  8 ways (~4.6MB/core), unlike batch sharding which would replicate it.
- Host-side im2col + layout prep (numpy, pure data movement): both the
  weight and the extracted windows are laid out per-core as
  [96 partitions = (i,k)-chunk, free = loc-major (loc, chunk, {o|b})]
  so device DMAs are large contiguous transfers.
- On device, per location: out[o, b] += wT_chunk[ik, o].T @ win_chunk[ik, b]
  with the 288-deep (i,k) contraction split as 3 chunks of 96 partitions,
  accumulated in PSUM. fp32 end-to-end (error ~1e-6 vs fp32 reference).
"""

import os
import numpy as np

import concourse.bacc as bacc
import concourse.mybir as mybir
import concourse.tile as tile
from concourse.bass_utils import run_bass_kernel_spmd

# Problem shapes (hardcoded per contract).
B, CI, H, W = 8, 32, 64, 64
CO = 32
KH = KW = 3
DH = DW = 2
HO = WO = 31
N_CORES = 8
RPC = 4                 # padded H'-rows per core
L = RPC * WO            # 124 locations per core
IK = CI * KH * KW       # 288 contraction
NCHUNK = 3
CK = IK // NCHUNK       # 96 partitions per chunk
GROUPS = RPC            # one compute/DMA group per H'-row
GL = L // GROUPS        # 31 locations per group

W_COLS = L * NCHUNK * CO     # 11904
WIN_COLS = L * NCHUNK * B    # 2976
OUT_COLS = L * B             # 992

_ROWS_PADDED = [[min(4 * c + j, HO - 1) for j in range(RPC)] for c in range(N_CORES)]

_NC_CACHE = {}


def _build_nc():
    nc = bacc.Bacc("TRN2", target_bir_lowering=False)
    wT = nc.dram_tensor("wT", [CK, W_COLS], mybir.dt.float32, kind="ExternalInput")
    winT = nc.dram_tensor("winT", [CK, WIN_COLS], mybir.dt.float32, kind="ExternalInput")
    out = nc.dram_tensor("out", [CO, OUT_COLS], mybir.dt.float32, kind="ExternalOutput")

    gw = GL * NCHUNK * CO    # weight cols per group
    gwin = GL * NCHUNK * B   # window cols per group
    gout = GL * B            # out cols per group

    with tile.TileContext(nc) as tc:
        with (
            tc.tile_pool(name="wp", bufs=3) as wp,
            tc.tile_pool(name="winp", bufs=3) as winp,
            tc.tile_pool(name="pp", bufs=2, space="PSUM") as pp,
            tc.tile_pool(name="op", bufs=2) as op,
        ):
            for g in range(GROUPS):
                wt = wp.tile([CK, gw], mybir.dt.float32, tag="wt")
                nc.sync.dma_start(wt[:], wT.ap()[:, g * gw:(g + 1) * gw])
                wint = winp.tile([CK, gwin], mybir.dt.float32, tag="wint")
                nc.sync.dma_start(wint[:], winT.ap()[:, g * gwin:(g + 1) * gwin])

                ps = pp.tile([CO, gout], mybir.dt.float32, tag="ps")
                for l in range(GL):
                    for c in range(NCHUNK):
                        nc.tensor.matmul(
                            ps[:, l * B:(l + 1) * B],
                            lhsT=wt[:, (l * NCHUNK + c) * CO:(l * NCHUNK + c + 1) * CO],
                            rhs=wint[:, (l * NCHUNK + c) * B:(l * NCHUNK + c + 1) * B],
                            start=(c == 0),
                            stop=(c == NCHUNK - 1),
                        )

                ot = op.tile([CO, gout], mybir.dt.float32, tag="ot")
                nc.any.tensor_copy(ot[:], ps[:])
                nc.sync.dma_start(out.ap()[:, g * gout:(g + 1) * gout], ot[:])
    nc.compile()
    return nc


def _host_prep(x, weight):
    """Build per-core DMA-ready layouts. Pure indexing/transpose, no math."""
    x = np.ascontiguousarray(np.asarray(x, dtype=np.float32))
    weight = np.ascontiguousarray(np.asarray(weight, dtype=np.float32))

    # windows[b, i, p, q, k] with k = kh*3+kw (matches torch unfold flatten)
    wins = np.stack(
        [x[:, :, kh:kh + DH * HO:DH, kw:kw + DW * WO:DW]
         for kh in range(KH) for kw in range(KW)],
        axis=-1,
    )  # (B, CI, HO, WO, 9)

    # (ik, p, q, o) and (ik, p, q, b)
    W2 = weight[0].transpose(1, 4, 2, 3, 0).reshape(IK, HO, WO, CO)
    W3 = wins.transpose(1, 4, 2, 3, 0).reshape(IK, HO, WO, B)

    in_maps = []
    for c in range(N_CORES):
        rows = _ROWS_PADDED[c]
        wsel = W2[:, rows].reshape(IK, L, CO)
        winsel = W3[:, rows].reshape(IK, L, B)
        # [CK, loc, chunk, {o|b}] — partition r of chunk-c col region holds ik=96c+r
        wT = np.stack([wsel[CK * cc:CK * (cc + 1)] for cc in range(NCHUNK)], axis=2)
        winT = np.stack([winsel[CK * cc:CK * (cc + 1)] for cc in range(NCHUNK)], axis=2)
        in_maps.append({
            "wT": np.ascontiguousarray(wT.reshape(CK, W_COLS)),
            "winT": np.ascontiguousarray(winT.reshape(CK, WIN_COLS)),
        })
    return in_maps


def _assemble(results):
    out = np.empty((B, CO, HO, WO), np.float32)
    for c in range(N_CORES):
        rr = np.asarray(results[c]["out"]).reshape(CO, RPC, WO, B)
        nreal = RPC if c < N_CORES - 1 else HO - 4 * (N_CORES - 1)
        for j in range(nreal):
            out[:, :, 4 * c + j, :] = rr[:, j, :, :].transpose(2, 0, 1)
    return out


def kernel(x, weight, _trace=False, _trace_cores=None):
    in_maps = _host_prep(x, weight)
    if "nc" not in _NC_CACHE:
        _NC_CACHE["nc"] = _build_nc()
    nc = _NC_CACHE["nc"]
    res = run_bass_kernel_spmd(
        nc, in_maps, core_ids=list(range(N_CORES)),
        trace=_trace, trace_cores=_trace_cores,
    )
    out = _assemble(res.results)
    if _trace:
        return out, res
    return out


if __name__ == "__main__":
    # quick self-check with random data against a numpy oracle
    rng = np.random.default_rng(0)
    x = rng.standard_normal((B, CI, H, W), dtype=np.float32)
    weight = rng.standard_normal((1, CO, CI, HO, WO, KH * KW), dtype=np.float32)
    wins = np.stack(
        [x[:, :, kh:kh + DH * HO:DH, kw:kw + DW * WO:DW]
         for kh in range(KH) for kw in range(KW)], axis=-1)
    expected = np.einsum("bipqk,oipqk->bopq", wins, weight[0], optimize=True)
    actual = kernel(x, weight)
    err = np.abs(actual - expected).max() / np.abs(expected).max()
    print("max out:", np.abs(expected).max(), "rel err:", err)
    assert err < 1e-5, err
    print("KERNEL OK")


# revision 2
# speedup vs baseline: 2.7731x; 2.7731x over previous
"""LocallyConnected2d Trainium2 kernel.

out[b,o,p,q] = sum_{i,kh,kw} x[b, i, 2p+kh, 2q+kw] * weight[0, o, i, p, q, kh*3+kw]

Strategy:
- Shard the H' (=31) output-row dimension across 8 cores (4 rows/core,
  core 7 gets a duplicated padding row so the SPMD program is uniform).
  This splits the dominant traffic — the 35.4MB per-location weight —
  8 ways (~4.6MB/core), unlike batch sharding which would replicate it.
- Host-side im2col + layout prep (numpy, pure data movement): both the
  weight and the extracted windows are laid out per-core as
  [96 partitions = (i,k)-chunk, free = loc-major (loc, chunk, {o|b})]
  so device DMAs are large contiguous transfers.
- On device, per location: out[o, b] += wT_chunk[ik, o].T @ win_chunk[ik, b]
  with the 288-deep (i,k) contraction split as 3 chunks of 96 partitions,
  accumulated in PSUM. fp32 end-to-end (error ~1e-6 vs fp32 reference).
"""

import os
import numpy as np

import concourse.bacc as bacc
import concourse.mybir as mybir
import concourse.tile as tile
from concourse.bass_utils import run_bass_kernel_spmd

# Problem shapes (hardcoded per contract).
B, CI, H, W = 8, 32, 64, 64
CO = 32
KH = KW = 3
DH = DW = 2
HO = WO = 31
N_CORES = 8
RPC = 4                 # padded H'-rows per core
L = RPC * WO            # 124 locations per core
IK = CI * KH * KW       # 288 contraction
NCHUNK = 3
CK = IK // NCHUNK       # 96 partitions per chunk
GROUPS = RPC            # one compute/DMA group per H'-row
GL = L // GROUPS        # 31 locations per group

W_COLS = L * NCHUNK * CO     # 11904
WIN_COLS = L * NCHUNK * B    # 2976
OUT_COLS = L * B             # 992

_ROWS_PADDED = [[min(4 * c + j, HO - 1) for j in range(RPC)] for c in range(N_CORES)]

_NC_CACHE = {}


def _build_nc(repeat=1):
    nc = bacc.Bacc("TRN2", target_bir_lowering=False)
    wT = nc.dram_tensor("wT", [CK, W_COLS], mybir.dt.float32, kind="ExternalInput")
    winT = nc.dram_tensor("winT", [CK, WIN_COLS], mybir.dt.float32, kind="ExternalInput")
    out = nc.dram_tensor("out", [CO, OUT_COLS], mybir.dt.float32, kind="ExternalOutput")

    gw = GL * NCHUNK * CO    # weight cols per group
    gwin = GL * NCHUNK * B   # window cols per group
    gout = GL * B            # out cols per group

    with tile.TileContext(nc) as tc:
        with (
            tc.tile_pool(name="wp", bufs=3) as wp,
            tc.tile_pool(name="winp", bufs=3) as winp,
            tc.tile_pool(name="pp", bufs=2, space="PSUM") as pp,
            tc.tile_pool(name="op", bufs=2) as op,
        ):
            def body():
                for g in range(GROUPS):
                    wt = wp.tile([CK, gw], mybir.dt.float32, tag="wt", name="wt")
                    nc.sync.dma_start(wt[:], wT.ap()[:, g * gw:(g + 1) * gw])
                    wint = winp.tile([CK, gwin], mybir.dt.float32, tag="wint", name="wint")
                    nc.sync.dma_start(wint[:], winT.ap()[:, g * gwin:(g + 1) * gwin])

                    ps = pp.tile([CO, gout], mybir.dt.float32, tag="ps", name="ps")
                    for l in range(GL):
                        for c in range(NCHUNK):
                            nc.tensor.matmul(
                                ps[:, l * B:(l + 1) * B],
                                lhsT=wt[:, (l * NCHUNK + c) * CO:(l * NCHUNK + c + 1) * CO],
                                rhs=wint[:, (l * NCHUNK + c) * B:(l * NCHUNK + c + 1) * B],
                                start=(c == 0),
                                stop=(c == NCHUNK - 1),
                            )

                    ot = op.tile([CO, gout], mybir.dt.float32, tag="ot", name="ot")
                    nc.any.tensor_copy(ot[:], ps[:])
                    nc.sync.dma_start(out.ap()[:, g * gout:(g + 1) * gout], ot[:])

            if repeat == 1:
                body()
            else:
                with tc.For_i(0, repeat, 1):
                    body()
    nc.compile()
    return nc


def _host_prep(x, weight):
    """Build per-core DMA-ready layouts. Pure indexing/transpose, no math."""
    x = np.ascontiguousarray(np.asarray(x, dtype=np.float32))
    weight = np.ascontiguousarray(np.asarray(weight, dtype=np.float32))

    # windows[b, i, p, q, k] with k = kh*3+kw (matches torch unfold flatten)
    wins = np.stack(
        [x[:, :, kh:kh + DH * HO:DH, kw:kw + DW * WO:DW]
         for kh in range(KH) for kw in range(KW)],
        axis=-1,
    )  # (B, CI, HO, WO, 9)

    # (ik, p, q, o) and (ik, p, q, b)
    W2 = weight[0].transpose(1, 4, 2, 3, 0).reshape(IK, HO, WO, CO)
    W3 = wins.transpose(1, 4, 2, 3, 0).reshape(IK, HO, WO, B)

    in_maps = []
    for c in range(N_CORES):
        rows = _ROWS_PADDED[c]
        wsel = W2[:, rows].reshape(IK, L, CO)
        winsel = W3[:, rows].reshape(IK, L, B)
        # [CK, loc, chunk, {o|b}] — partition r of chunk-c col region holds ik=96c+r
        wT = np.stack([wsel[CK * cc:CK * (cc + 1)] for cc in range(NCHUNK)], axis=2)
        winT = np.stack([winsel[CK * cc:CK * (cc + 1)] for cc in range(NCHUNK)], axis=2)
        in_maps.append({
            "wT": np.ascontiguousarray(wT.reshape(CK, W_COLS)),
            "winT": np.ascontiguousarray(winT.reshape(CK, WIN_COLS)),
        })
    return in_maps


def _assemble(results):
    out = np.empty((B, CO, HO, WO), np.float32)
    for c in range(N_CORES):
        rr = np.asarray(results[c]["out"]).reshape(CO, RPC, WO, B)
        nreal = RPC if c < N_CORES - 1 else HO - 4 * (N_CORES - 1)
        for j in range(nreal):
            out[:, :, 4 * c + j, :] = rr[:, j, :, :].transpose(2, 0, 1)
    return out


def kernel(x, weight, _trace=False, _trace_cores=None):
    in_maps = _host_prep(x, weight)
    if "nc" not in _NC_CACHE:
        _NC_CACHE["nc"] = _build_nc()
    nc = _NC_CACHE["nc"]
    res = run_bass_kernel_spmd(
        nc, in_maps, core_ids=list(range(N_CORES)),
        trace=_trace, trace_cores=_trace_cores,
    )
    out = _assemble(res.results)
    if _trace:
        return out, res
    return out


if __name__ == "__main__":
    # quick self-check with random data against a numpy oracle
    rng = np.random.default_rng(0)
    x = rng.standard_normal((B, CI, H, W), dtype=np.float32)
    weight = rng.standard_normal((1, CO, CI, HO, WO, KH * KW), dtype=np.float32)
    wins = np.stack(
        [x[:, :, kh:kh + DH * HO:DH, kw:kw + DW * WO:DW]
         for kh in range(KH) for kw in range(KW)], axis=-1)
    expected = np.einsum("bipqk,oipqk->bopq", wins, weight[0], optimize=True)
    actual = kernel(x, weight)
    err = np.abs(actual - expected).max() / np.abs(expected).max()
    print("max out:", np.abs(expected).max(), "rel err:", err)
    assert err < 1e-5, err
    print("KERNEL OK")


# revision 5
# speedup vs baseline: 4.1763x; 1.5060x over previous
"""LocallyConnected2d Trainium2 kernel.

out[b,o,p,q] = sum_{i,kh,kw} x[b, i, 2p+kh, 2q+kw] * weight[0, o, i, p, q, kh*3+kw]

Strategy:
- Shard the H' (=31) output-row dimension across 8 cores (4 rows/core,
  core 7 gets a duplicated padding row so the SPMD program is uniform).
  This splits the dominant traffic — the 35.4MB per-location weight —
  8 ways (~4.6MB/core), unlike batch sharding which would replicate it.
- Host-side im2col + layout prep (numpy, pure data movement): both the
  weight and the extracted windows are laid out per-core as
  [96 partitions = (i,k)-chunk, free = loc-major (loc, chunk, {o|b})]
  so device DMAs are large contiguous transfers.
- On device, per location: out[o, b] += wT_chunk[ik, o].T @ win_chunk[ik, b]
  with the 288-deep (i,k) contraction split as 3 chunks of 96 partitions,
  accumulated in PSUM. fp32 end-to-end (error ~1e-6 vs fp32 reference).
"""

import os
import numpy as np

import concourse.bacc as bacc
import concourse.mybir as mybir
import concourse.tile as tile
from concourse.bass_utils import run_bass_kernel_spmd

# Problem shapes (hardcoded per contract).
B, CI, H, W = 8, 32, 64, 64
CO = 32
KH = KW = 3
DH = DW = 2
HO = WO = 31
N_CORES = 8
RPC = 4                 # padded H'-rows per core
L = RPC * WO            # 124 locations per core
IK = CI * KH * KW       # 288 contraction
NCHUNK = 3
CK = IK // NCHUNK       # 96 partitions per chunk
GROUPS = RPC            # one compute/DMA group per H'-row
GL = L // GROUPS        # 31 locations per group

W_COLS = L * NCHUNK * CO     # 11904
WIN_COLS = L * NCHUNK * B    # 2976
OUT_COLS = L * B             # 992

_ROWS_PADDED = [[min(4 * c + j, HO - 1) for j in range(RPC)] for c in range(N_CORES)]

_NC_CACHE = {}


V2_GOUT = 256               # psum cols per group in v2: 8 col-blocks x 32 (o)
V2_OUT_COLS = V2_GOUT * GROUPS


def _build_nc(repeat=1, variant="v2"):
    nc = bacc.Bacc("TRN2", target_bir_lowering=False)
    wT = nc.dram_tensor("wT", [CK, W_COLS], mybir.dt.float32, kind="ExternalInput")
    winT = nc.dram_tensor("winT", [CK, WIN_COLS], mybir.dt.float32, kind="ExternalInput")
    out_cols = OUT_COLS if variant == "v1" else V2_OUT_COLS
    out_rows = CO if variant == "v1" else 128
    out = nc.dram_tensor("out", [out_rows, out_cols], mybir.dt.float32, kind="ExternalOutput")

    gw = GL * NCHUNK * CO    # weight cols per group
    gwin = GL * NCHUNK * B   # window cols per group
    gout = GL * B            # v1 out cols per group

    with tile.TileContext(nc) as tc:
        with (
            tc.tile_pool(name="wp", bufs=3) as wp,
            tc.tile_pool(name="winp", bufs=3) as winp,
            tc.tile_pool(name="pp", bufs=2, space="PSUM") as pp,
            tc.tile_pool(name="op", bufs=2) as op,
        ):
            def body_v1():
                for g in range(GROUPS):
                    wt = wp.tile([CK, gw], mybir.dt.float32, tag="wt", name="wt")
                    nc.sync.dma_start(wt[:], wT.ap()[:, g * gw:(g + 1) * gw])
                    wint = winp.tile([CK, gwin], mybir.dt.float32, tag="wint", name="wint")
                    nc.sync.dma_start(wint[:], winT.ap()[:, g * gwin:(g + 1) * gwin])

                    ps = pp.tile([CO, gout], mybir.dt.float32, tag="ps", name="ps")
                    for l in range(GL):
                        for c in range(NCHUNK):
                            nc.tensor.matmul(
                                ps[:, l * B:(l + 1) * B],
                                lhsT=wt[:, (l * NCHUNK + c) * CO:(l * NCHUNK + c + 1) * CO],
                                rhs=wint[:, (l * NCHUNK + c) * B:(l * NCHUNK + c + 1) * B],
                                start=(c == 0),
                                stop=(c == NCHUNK - 1),
                            )

                    ot = op.tile([CO, gout], mybir.dt.float32, tag="ot", name="ot")
                    nc.any.tensor_copy(ot[:], ps[:])
                    nc.sync.dma_start(out.ap()[:, g * gout:(g + 1) * gout], ot[:])

            def body_v2():
                # stationary = windows (8 cols, cheap fp32 self-load);
                # moving = weight (N=32); out[b, o] block at partition
                # offset 32*(l%4) via col-tiling -> 4 concurrent MM strips.
                for g in range(GROUPS):
                    wt = wp.tile([CK, gw], mybir.dt.float32, tag="wt", name="wt")
                    nc.sync.dma_start(wt[:], wT.ap()[:, g * gw:(g + 1) * gw])
                    wint = winp.tile([CK, gwin], mybir.dt.float32, tag="wint", name="wint")
                    nc.sync.dma_start(wint[:], winT.ap()[:, g * gwin:(g + 1) * gwin])

                    ps = pp.tile([128, V2_GOUT], mybir.dt.float32, tag="ps", name="ps")
                    for l in range(GL):
                        j = l % 4
                        blk = l // 4
                        for c in range(NCHUNK):
                            nc.tensor.matmul(
                                ps[32 * j:32 * j + B, blk * CO:(blk + 1) * CO],
                                lhsT=wint[:, (l * NCHUNK + c) * B:(l * NCHUNK + c + 1) * B],
                                rhs=wt[:, (l * NCHUNK + c) * CO:(l * NCHUNK + c + 1) * CO],
                                start=(c == 0),
                                stop=(c == NCHUNK - 1),
                                tile_position=(0, 32 * j),
                            )

                    ot = op.tile([128, V2_GOUT], mybir.dt.float32, tag="ot", name="ot")
                    nc.any.tensor_copy(ot[:], ps[:])
                    nc.sync.dma_start(out.ap()[:, g * V2_GOUT:(g + 1) * V2_GOUT], ot[:])

            body = body_v1 if variant == "v1" else body_v2
            if repeat == 1:
                body()
            else:
                with tc.For_i(0, repeat, 1):
                    body()
    nc.compile()
    return nc


def _host_prep(x, weight):
    """Build per-core DMA-ready layouts. Pure indexing/transpose, no math."""
    x = np.ascontiguousarray(np.asarray(x, dtype=np.float32))
    weight = np.ascontiguousarray(np.asarray(weight, dtype=np.float32))

    # windows[b, i, p, q, k] with k = kh*3+kw (matches torch unfold flatten)
    wins = np.stack(
        [x[:, :, kh:kh + DH * HO:DH, kw:kw + DW * WO:DW]
         for kh in range(KH) for kw in range(KW)],
        axis=-1,
    )  # (B, CI, HO, WO, 9)

    # (ik, p, q, o) and (ik, p, q, b)
    W2 = weight[0].transpose(1, 4, 2, 3, 0).reshape(IK, HO, WO, CO)
    W3 = wins.transpose(1, 4, 2, 3, 0).reshape(IK, HO, WO, B)

    in_maps = []
    for c in range(N_CORES):
        rows = _ROWS_PADDED[c]
        wsel = W2[:, rows].reshape(IK, L, CO)
        winsel = W3[:, rows].reshape(IK, L, B)
        # [CK, loc, chunk, {o|b}] — partition r of chunk-c col region holds ik=96c+r
        wT = np.stack([wsel[CK * cc:CK * (cc + 1)] for cc in range(NCHUNK)], axis=2)
        winT = np.stack([winsel[CK * cc:CK * (cc + 1)] for cc in range(NCHUNK)], axis=2)
        in_maps.append({
            "wT": np.ascontiguousarray(wT.reshape(CK, W_COLS)),
            "winT": np.ascontiguousarray(winT.reshape(CK, WIN_COLS)),
        })
    return in_maps


def _assemble(results, variant="v2"):
    out = np.empty((B, CO, HO, WO), np.float32)
    qs = np.arange(WO)
    for c in range(N_CORES):
        nreal = RPC if c < N_CORES - 1 else HO - 4 * (N_CORES - 1)
        buf = np.asarray(results[c]["out"])
        if variant == "v1":
            rr = buf.reshape(CO, RPC, WO, B)
            for j in range(nreal):
                out[:, :, 4 * c + j, :] = rr[:, j, :, :].transpose(2, 0, 1)
        else:
            # buf [128, GROUPS*256]: row = 32*(q%4)+b, col = g*256+(q//4)*32+o
            b4 = buf.reshape(4, 32, GROUPS, 8, CO)
            res = b4[qs % 4, :B, :, qs // 4, :]      # (31, b, g, o)
            out[:, :, 4 * c:4 * c + nreal, :] = res.transpose(1, 3, 2, 0)[:, :, :nreal, :]
    return out


VARIANT = os.environ.get("LC2D_VARIANT", "v2")


def kernel(x, weight, _trace=False, _trace_cores=None):
    in_maps = _host_prep(x, weight)
    if "nc" not in _NC_CACHE:
        _NC_CACHE["nc"] = _build_nc(variant=VARIANT)
    nc = _NC_CACHE["nc"]
    res = run_bass_kernel_spmd(
        nc, in_maps, core_ids=list(range(N_CORES)),
        trace=_trace, trace_cores=_trace_cores,
    )
    out = _assemble(res.results, variant=VARIANT)
    if _trace:
        return out, res
    return out


if __name__ == "__main__":
    # quick self-check with random data against a numpy oracle
    rng = np.random.default_rng(0)
    x = rng.standard_normal((B, CI, H, W), dtype=np.float32)
    weight = rng.standard_normal((1, CO, CI, HO, WO, KH * KW), dtype=np.float32)
    wins = np.stack(
        [x[:, :, kh:kh + DH * HO:DH, kw:kw + DW * WO:DW]
         for kh in range(KH) for kw in range(KW)], axis=-1)
    expected = np.einsum("bipqk,oipqk->bopq", wins, weight[0], optimize=True)
    actual = kernel(x, weight)
    err = np.abs(actual - expected).max() / np.abs(expected).max()
    print("max out:", np.abs(expected).max(), "rel err:", err)
    assert err < 1e-5, err
    print("KERNEL OK")
